# revision 1
# baseline (speedup 1.0000x reference)
"""Trainium2 Bass kernel for MatrixFactorizationIF (embedding-lookup style).

Computation per batch element b with indices (i, j, k):
    out[b] = ALPHA * <pF[i, :64], M[j]>
           + BETA^2 * sum_s <Vs[i,:,s], M[j]> * <Vg[i,:,s], M[k]>

Strategy (cost-model driven; ~2.7-3.7x faster than the f32 gather baseline):
  - pF repacked host-side into 512-byte rows: [Pi*ALPHA bf16 | Vs fp8 s-major
    | Vg fp8 s-major]; 512-B gather descriptors run at full modeled DMA rate.
  - M repacked to bf16 [N_M, 64]; gathered with 128-B descriptors
    (raw-emitted InstDMAGatherAnt; the 256-B bass assert is transpose-only).
  - Batch sharded by i-range; elements sorted into 16 groups by (j%4, k%4)
    for int16-safe strided M views.  Group capacity C is computed from the
    actual inputs and compiled in (cached per C).
  - fp8 V upcast to bf16 on ACT.  Per-element compute: products + log-tree
    dot reductions in one wide [T,7,128] bf16 tile (products cols 0:64, tree
    levels 64:96:112:120:124:126:127).  DVE handles most columns (bf16 2x);
    the Pool engine (0.42-derated) takes the last TP columns per group to
    shave the DVE critical path.
  - Gather emission runs one group ahead of compute (software pipelining) so
    Pool DGE keeps the DMA device fed while Pool/DVE compute the prior group.
"""

import numpy as np
import ml_dtypes

# Problem constants (hardcoded per the harness contract).
N_P = 100000
N_M = 100000
R = 64
S = 3
B = 500000
ALPHA = 0.001
BETA = 0.001

N_CORES = 8
P = 128
PF_SHARD = N_P // N_CORES   # 12500 pF rows per core
NG = 16                     # groups per core: (j%4, k%4)
GSZ = 8192                  # max indices per dma_gather call
EQ = 256                    # packed pF row: 256 int16 = 512 B
TP = 5                      # columns per group computed on Pool (steady)
TP_ALT = 6                  # alternating higher split
TPS = [0] * 16  # Pool vector ops are heavily derated on HW; keep compute on DVE


def _round_up(x, m):
    return -(-x // m) * m


def _raw_gather(g, mybir, out_ap, in_ap, idxs_ap, num_idxs, elem_size,
                elem_step, queue_num, reg=None):
    """dma_gather without the elem_size_bytes%256 assert (non-transpose)."""
    from concourse import ap_utils
    from concourse.bass import MemorySpace, exact_div
    assert idxs_ap.dtype == mybir.dt.int16
    assert in_ap.dtype == out_ap.dtype
    assert in_ap.space == MemorySpace.DRAM
    assert ap_utils.ap_is_contiguous(out_ap.ap[1:])
    assert ap_utils.ap_is_contiguous(idxs_ap.ap[1:])
    assert in_ap.ap[-1][1] == out_ap.ap[-1][1] == elem_size
    assert in_ap.ap[0][0] == elem_step
    stride_bytes = elem_step * mybir.dt.size(in_ap.dtype)
    return g.add_instruction(
        mybir.InstDMAGatherAnt(
            name=g.bass.get_next_instruction_name(),
            ins=[*g.lower_ap_dma(in_ap, for_custom_bir_dma=True),
                 g.lower_ap(idxs_ap),
                 g.lower_val_access(reg if reg is not None
                                    else g.to_reg(num_idxs))],
            outs=[g.lower_ap(out_ap)],
            transpose=False,
            num_idxs=num_idxs,
            elem_size=elem_size,
            stride_bytes_256=exact_div(stride_bytes, 256),
            gen_mode=0,
            single_packet=False,
            queue_num=queue_num,
            sbuf_tokens_per_rank=0,
            sbuf_free_dim_per_rank=0,
            sbuf_free_dim_pad_per_rank=0,
            sbuf_byte_offset=0,
        ))


def build_program(C, repeat=1, q3=True):
    """Build the per-core program for group capacity C (multiple of 128)."""
    import concourse.bass as bass
    import concourse.bacc as bacc
    import concourse.mybir as mybir
    from concourse.tile import TileContext

    q1, q2 = (1, 2) if q3 else (0, 0)
    f32 = mybir.dt.float32
    bf16 = mybir.dt.bfloat16
    i16 = mybir.dt.int16
    f8 = mybir.dt.float8e4
    mult = mybir.AluOpType.mult
    add = mybir.AluOpType.add
    AX = mybir.AxisListType.X

    T = C // P          # free columns per tile
    TD = T - TP         # columns on DVE
    C16 = C // 16       # wrapped index columns per stream

    nc = bacc.Bacc("TRN2", target_bir_lowering=False, num_swdge_queues=3)
    pFq = nc.dram_tensor("pFq", [PF_SHARD, EQ], i16, kind="ExternalInput")
    Mb = nc.dram_tensor("Mb", [N_M, R], bf16, kind="ExternalInput")
    idx = nc.dram_tensor("idx", [P, NG * 3 * C16], i16, kind="ExternalInput")
    out = nc.dram_tensor("out", [NG * C], f32, kind="ExternalOutput")

    with TileContext(nc) as tc:
        with (
            tc.tile_pool(name="idx", bufs=1) as idx_pool,
            tc.tile_pool(name="pf", bufs=3) as pf_pool,
            tc.tile_pool(name="m", bufs=3) as m_pool,
            tc.tile_pool(name="vq", bufs=2) as vq_pool,
            tc.tile_pool(name="prod", bufs=1) as prod_pool,
            tc.tile_pool(name="small", bufs=1) as small_pool,
            tc.tile_pool(name="res", bufs=2) as res_pool,
        ):
            creg = nc.gpsimd.to_reg(C)
            idx_state = {}

            def emit_gathers(it, gsz=GSZ):
                gl = it % NG
                jc, kc = gl >> 2, gl & 3
                ib = gl * 3 * C16

                if it % NG == 0:
                    idx_big = idx_pool.tile([P, NG * 3 * C16], i16)
                    nc.sync.dma_start(out=idx_big[:], in_=idx[:])
                    idx_state["t"] = idx_big
                idx_t = idx_state["t"][:]

                pf_t = pf_pool.tile([P, T * EQ], i16)
                mj_t = m_pool.tile([P, T * R], bf16, tag="mj")
                mk_t = m_pool.tile([P, T * R], bf16, tag="mk")
                pf4 = pf_t[:].rearrange("p (t e) -> p t e", e=EQ)
                mj3 = mj_t[:].rearrange("p (t r) -> p t r", r=R)
                mk3 = mk_t[:].rearrange("p (t r) -> p t r", r=R)

                mjview = Mb[:].rearrange(
                    "(n f) r -> n (f r)", f=4)[:, jc * R:(jc + 1) * R]
                mkview = Mb[:].rearrange(
                    "(n f) r -> n (f r)", f=4)[:, kc * R:(kc + 1) * R]

                for off in range(0, C, gsz):
                    n = min(gsz, C - off)
                    oc0, ocn = off // P, n // P
                    ic0, icn = off // 16, n // 16
                    nc.gpsimd.dma_gather(
                        out_ap=pf4[:, oc0:oc0 + ocn, :],
                        in_ap=pFq[:],
                        idxs_ap=idx_t[:, ib + ic0:ib + ic0 + icn],
                        num_idxs=n, num_idxs_reg=creg if n == C else n,
                        elem_size=EQ,
                        single_packet=n <= 1024, queue_num=0)
                    _raw_gather(
                        nc.gpsimd, mybir,
                        out_ap=mj3[:, oc0:oc0 + ocn, :],
                        in_ap=mjview,
                        idxs_ap=idx_t[:, ib + C16 + ic0:ib + C16 + ic0 + icn],
                        num_idxs=n, elem_size=R, elem_step=4 * R,
                        queue_num=q1, reg=creg if n == C else None)
                    _raw_gather(
                        nc.gpsimd, mybir,
                        out_ap=mk3[:, oc0:oc0 + ocn, :],
                        in_ap=mkview,
                        idxs_ap=idx_t[:, ib + 2 * C16 + ic0:ib + 2 * C16 + ic0 + icn],
                        num_idxs=n, elem_size=R, elem_step=4 * R,
                        queue_num=q2, reg=creg if n == C else None)
                return pf4, mj3, mk3

            def emit_half(eng, pw_t, tloc, pf4, mj3, mkq3, vq5, c0, cn, gl,
                          tag):
                """Products + tree + tail for columns [c0:c0+cn) on engine
                `eng` (nc.vector or nc.gpsimd)."""
                pw = pw_t[:, 0:cn * 7 * 128].rearrange(
                    "p (t s e) -> p t s e", s=7, e=128)
                pf = pf4[:, c0:c0 + cn]
                mjb = mj3[:, c0:c0 + cn, None, :].to_broadcast(
                    [P, cn, S, R])
                mkb = mkq3[:, c0:c0 + cn, None, :].to_broadcast(
                    [P, cn, S, R])
                pi = pf[:, :, 0:64].bitcast(bf16)
                eng.tensor_tensor(
                    out=pw[:, :, 0, 0:64], in0=pi, in1=mj3[:, c0:c0 + cn],
                    op=mult)
                eng.tensor_tensor(
                    out=pw[:, :, 1:4, 0:64], in0=vq5[:, :, 0:3, :], in1=mjb,
                    op=mult)
                eng.tensor_tensor(
                    out=pw[:, :, 4:7, 0:64], in0=vq5[:, :, 3:6, :], in1=mkb,
                    op=mult)
                eng.tensor_tensor(
                    out=pw[:, :, :, 64:96], in0=pw[:, :, :, 0:32],
                    in1=pw[:, :, :, 32:64], op=add)
                eng.tensor_tensor(
                    out=pw[:, :, :, 96:112], in0=pw[:, :, :, 64:80],
                    in1=pw[:, :, :, 80:96], op=add)
                eng.tensor_tensor(
                    out=pw[:, :, :, 112:120], in0=pw[:, :, :, 96:104],
                    in1=pw[:, :, :, 104:112], op=add)
                eng.tensor_tensor(
                    out=pw[:, :, :, 120:124], in0=pw[:, :, :, 112:116],
                    in1=pw[:, :, :, 116:120], op=add)
                eng.tensor_tensor(
                    out=pw[:, :, :, 124:126], in0=pw[:, :, :, 120:122],
                    in1=pw[:, :, :, 122:124], op=add)
                eng.tensor_tensor(
                    out=pw[:, :, :, 126:127], in0=pw[:, :, :, 124:125],
                    in1=pw[:, :, :, 125:126], op=add)

                agp_t = small_pool.tile([P, cn * S], f32, tag="agp" + tag)
                agp = agp_t[:].rearrange("p (t s) -> p t s", s=S)
                eng.tensor_tensor(
                    out=agp, in0=pw[:, :, 1:4, 126], in1=pw[:, :, 4:7, 126],
                    op=mult)
                ags = small_pool.tile([P, cn], f32, tag="ags" + tag)
                eng.reduce_sum(out=ags[:], in_=agp, axis=AX)
                res = res_pool.tile([P, cn], f32, tag="res" + tag)
                eng.scalar_tensor_tensor(
                    out=res[:], in0=ags[:], scalar=BETA * BETA,
                    in1=pw[:, :, 0, 126], op0=mult, op1=add)
                nc.sync.dma_start(
                    out=out[gl * C + c0 * P:gl * C + (c0 + cn) * P].rearrange(
                        "(t p) -> p t", p=P),
                    in_=res[:])

            def emit_pool_half(pf4, mj3, mkq3, vq5, c0, cn, gl):
                """Pool half: TT products + TT-tree (walrus allows only
                InstTensorTensor on Pool); the final combine runs on DVE."""
                pvp_t = prod_pool.tile([P, cn * 7 * 128], bf16, tag="pvp")
                pw = pvp_t[:].rearrange("p (t s e) -> p t s e", s=7, e=128)
                pi = pf4[:, c0:c0 + cn, 0:64].bitcast(bf16)
                mjb = mj3[:, c0:c0 + cn, None, :].to_broadcast(
                    [P, cn, S, R])
                mkb = mkq3[:, c0:c0 + cn, None, :].to_broadcast(
                    [P, cn, S, R])
                g = nc.gpsimd
                g.tensor_tensor(
                    out=pw[:, :, 0, 0:64], in0=pi, in1=mj3[:, c0:c0 + cn],
                    op=mult)
                g.tensor_tensor(
                    out=pw[:, :, 1:4, 0:64], in0=vq5[:, :, 0:3, :], in1=mjb,
                    op=mult)
                g.tensor_tensor(
                    out=pw[:, :, 4:7, 0:64], in0=vq5[:, :, 3:6, :], in1=mkb,
                    op=mult)
                g.tensor_tensor(
                    out=pw[:, :, :, 64:96], in0=pw[:, :, :, 0:32],
                    in1=pw[:, :, :, 32:64], op=add)
                g.tensor_tensor(
                    out=pw[:, :, :, 96:112], in0=pw[:, :, :, 64:80],
                    in1=pw[:, :, :, 80:96], op=add)
                g.tensor_tensor(
                    out=pw[:, :, :, 112:120], in0=pw[:, :, :, 96:104],
                    in1=pw[:, :, :, 104:112], op=add)
                g.tensor_tensor(
                    out=pw[:, :, :, 120:124], in0=pw[:, :, :, 112:116],
                    in1=pw[:, :, :, 116:120], op=add)
                g.tensor_tensor(
                    out=pw[:, :, :, 124:126], in0=pw[:, :, :, 120:122],
                    in1=pw[:, :, :, 122:124], op=add)
                g.tensor_tensor(
                    out=pw[:, :, :, 126:127], in0=pw[:, :, :, 124:125],
                    in1=pw[:, :, :, 125:126], op=add)
                agp_t = small_pool.tile([P, cn * S], f32, tag="agpp")
                agp = agp_t[:].rearrange("p (t s) -> p t s", s=S)
                g.tensor_tensor(
                    out=agp, in0=pw[:, :, 1:4, 126], in1=pw[:, :, 4:7, 126],
                    op=mult)
                s1 = small_pool.tile([P, cn], f32, tag="s1p")
                g.tensor_tensor(
                    out=s1[:], in0=agp[:, :, 0], in1=agp[:, :, 1], op=add)
                ags = small_pool.tile([P, cn], f32, tag="agsp")
                g.tensor_tensor(
                    out=ags[:], in0=s1[:], in1=agp[:, :, 2], op=add)
                res = res_pool.tile([P, cn], f32, tag="resp")
                nc.vector.scalar_tensor_tensor(
                    out=res[:], in0=ags[:], scalar=BETA * BETA,
                    in1=pw[:, :, 0, 126], op0=mult, op1=add)
                nc.sync.dma_start(
                    out=out[gl * C + c0 * P:gl * C + (c0 + cn) * P].rearrange(
                        "(t p) -> p t", p=P),
                    in_=res[:])

            def emit_compute(it, tiles, first=False):
                gl = it % NG
                pf4, mj3, mk3 = tiles
                mkq3 = mk3
                vq_t = vq_pool.tile([P, T * 6 * R], bf16)
                vq4 = vq_t[:].rearrange(
                    "p (t s r) -> p t s r", s=2 * S, r=R)
                vq_flat = vq_t[:].rearrange("p (t e) -> p t e", e=6 * R)
                v8 = pf4[:, :, 64:256].bitcast(f8)

                if first:
                    # group 0: chunk columns so DVE starts ~15us earlier
                    tpmin0 = min(TPS) if TPS else min(TP, TP_ALT)
                    cw = 8
                    for c0 in range(0, T, cw):
                        nc.scalar.copy(
                            out=vq_flat[:, c0:c0 + cw], in_=v8[:, c0:c0 + cw])
                        pw0_t = prod_pool.tile(
                            [P, (T - tpmin0) * 7 * 128], bf16, tag="pwd")
                        emit_half(nc.vector, pw0_t, cw, pf4, mj3, mkq3,
                                  vq4[:, c0:c0 + cw], c0, cw, gl, "d")
                    return

                nc.scalar.copy(out=vq_flat, in_=v8)
                tp = TPS[it % NG] if TPS else (TP_ALT if (it % 2) else TP)
                td = T - tp
                tpmin = min(TPS) if TPS else min(TP, TP_ALT)
                pwd_t = prod_pool.tile(
                    [P, (T - tpmin) * 7 * 128], bf16, tag="pwd")
                emit_half(nc.vector, pwd_t, td, pf4, mj3, mkq3,
                          vq4[:, 0:td], 0, td, gl, "d")
                if tp > 0:
                    emit_pool_half(pf4, mj3, mkq3, vq4[:, td:T], td, tp, gl)

            tiles = {}
            for it in range(NG * repeat + 1):
                if it < NG * repeat:
                    gsz = 1024 if it == 0 else GSZ
                    tiles[it] = emit_gathers(it, gsz)
                if it >= 1:
                    emit_compute(it - 1, tiles.pop(it - 1),
                                 first=(it - 1 == 0))

    nc.compile()
    return nc


_NC_CACHE = {}


def _get_program(C, repeat=1):
    key = (C, repeat)
    if key not in _NC_CACHE:
        _NC_CACHE[key] = build_program(C, repeat)
    return _NC_CACHE[key]


def pack_tables(pF, M):
    """Host-side repack: pFq int16 [N_P, 256] (bf16 Pi*ALPHA | fp8 V s-major),
    Mb bf16 [N_M, 64]."""
    pi_b = (pF[:, :R] * ALPHA).astype(ml_dtypes.bfloat16)
    vs = pF[:, R:(1 + S) * R].reshape(N_P, R, S)
    vg = pF[:, (1 + S) * R:].reshape(N_P, R, S)
    v = np.concatenate(
        [vs.transpose(0, 2, 1).reshape(N_P, S * R),
         vg.transpose(0, 2, 1).reshape(N_P, S * R)], axis=1)
    v8 = v.astype(ml_dtypes.float8_e4m3)
    rows = np.concatenate(
        [pi_b.view(np.uint8).reshape(N_P, 2 * R),
         v8.view(np.uint8).reshape(N_P, 2 * S * R)], axis=1)
    pFq = np.ascontiguousarray(rows).view(np.int16)
    Mb = np.ascontiguousarray(M.astype(ml_dtypes.bfloat16))
    return pFq, Mb


def prepare_inputs(pF, M, ijk):
    """Host-side shard + sort + pad. Returns (in_maps, src_index, C)."""
    i = ijk[:, 0].astype(np.int64)
    j = ijk[:, 1].astype(np.int64)
    k = ijk[:, 2].astype(np.int64)

    core = i // PF_SHARD
    gl = (j & 3) * 4 + (k & 3)
    key = core * NG + gl
    order = np.argsort(key, kind="stable")
    counts = np.bincount(key, minlength=N_CORES * NG)
    C = max(128, _round_up(int(counts.max()), 128))
    starts = np.zeros(N_CORES * NG, np.int64)
    starts[1:] = np.cumsum(counts)[:-1]
    nb = len(i)
    rank = np.arange(nb) - np.repeat(starts, counts)
    rank_orig = np.empty(nb, np.int64)
    rank_orig[order] = rank
    src_index = core * (NG * C) + gl * C + rank_orig

    i_loc = (i - core * PF_SHARD).astype(np.int16)
    j_loc = (j >> 2).astype(np.int16)
    k_loc = (k >> 2).astype(np.int16)

    C16 = C // 16
    wrapped = np.zeros((N_CORES, NG, 3, 16, C16), np.int16)
    wp = (rank_orig % 16).astype(np.int64)
    ws = (rank_orig // 16).astype(np.int64)
    wrapped[core, gl, 0, wp, ws] = i_loc
    wrapped[core, gl, 1, wp, ws] = j_loc
    wrapped[core, gl, 2, wp, ws] = k_loc
    wrapped = np.tile(wrapped, (1, 1, 1, 8, 1))  # 16 -> 128 partitions
    wrapped = wrapped.reshape(N_CORES, NG * 3, 8, 16, C16).transpose(
        0, 2, 3, 1, 4).reshape(N_CORES, P, NG * 3 * C16)

    pFq, Mb = pack_tables(pF, M)
    in_maps = []
    for c in range(N_CORES):
        in_maps.append({
            "pFq": np.ascontiguousarray(pFq[c * PF_SHARD:(c + 1) * PF_SHARD]),
            "Mb": Mb,
            "idx": np.ascontiguousarray(wrapped[c]),
        })
    return in_maps, src_index, C


def kernel(pF, M, ijk):
    from concourse.bass_utils import run_bass_kernel_spmd

    pF = np.ascontiguousarray(np.asarray(pF, dtype=np.float32))
    M = np.ascontiguousarray(np.asarray(M, dtype=np.float32))
    ijk = np.asarray(ijk)

    in_maps, src_index, C = prepare_inputs(pF, M, ijk)
    nc = _get_program(C)

    results = run_bass_kernel_spmd(
        nc, in_maps, core_ids=list(range(N_CORES))).results

    flat = np.concatenate([results[c]["out"] for c in range(N_CORES)])
    return flat[src_index].astype(np.float32)

